# revision 17
# baseline (speedup 1.0000x reference)
"""Trainium2 Bass kernel for nn_Attn: softmax(enc @ (W^T h)) over seq_len.

Math: energy = enc @ W^T + b; attn = energy @ h; out = softmax(attn).
Algebraically attn[s] = enc[s,:] . v + (b.h) with v = W^T h, and the (b.h)
term is constant across s so softmax cancels it. The device work is the
memory-bound part: streaming the 128 MiB encoder_outputs once, sharded
along seq_len across 8 NeuronCores. Per 128-row block: VectorE multiplies
by v (tensor_tensor), ScalarE reduces rows (activation Copy + accum_out),
so the two passes over the data run on different engines concurrently.
"""
import numpy as np

S = 32768
H = 1024
N_CORES = 8
S_SHARD = S // N_CORES          # 4096 rows per core
P = 128                         # partitions
N_BLK = S_SHARD // P            # 32 row-blocks per core
# blocks per dma_start: small at the edges (fast pipeline rampup/drain),
# 2 MiB in the middle (DMA efficiency); must sum to N_BLK
DMA_SCHED = [1, 1] + [2] * 14 + [1, 1]
N_SPLIT_HEAD = 0                # first DMAs split 4-way across queues
E_CHUNKS = 4                    # output DMA'd in column chunks as it completes

_cache = {}


def _build():
    from concourse import bacc, mybir, tile

    nc = bacc.Bacc("TRN2", target_bir_lowering=False, debug=False,
                   num_devices=N_CORES)
    enc = nc.dram_tensor("enc", [S_SHARD, H], mybir.dt.float32,
                         kind="ExternalInput")
    v_in = nc.dram_tensor("v_in", [1, H], mybir.dt.float32,
                          kind="ExternalInput")
    e_out = nc.dram_tensor("e_out", [P, N_BLK], mybir.dt.float32,
                           kind="ExternalOutput")

    ECW = N_BLK // E_CHUNKS     # columns per output chunk

    with tile.TileContext(nc) as tc:
        with tc.tile_pool(name="const", bufs=1) as cpool, \
             tc.tile_pool(name="psum", bufs=1, space="PSUM") as qpool, \
             tc.tile_pool(name="stream", bufs=8) as spool, \
             tc.tile_pool(name="prod", bufs=4) as ppool, \
             tc.tile_pool(name="cpout", bufs=4) as opool:
            # broadcast v across partitions: vt = ones[P,1] @ v[1,H] on PE
            v0 = cpool.tile([1, H], mybir.dt.float32)
            nc.sync.dma_start(out=v0[:], in_=v_in.ap())
            ones = cpool.tile([1, P], mybir.dt.float32)
            nc.vector.memset(ones[:], 1.0)
            pv = qpool.tile([P, H], mybir.dt.float32)
            nc.tensor.matmul(out=pv[:, 0:512], lhsT=ones[:],
                             rhs=v0[:, 0:512], start=True, stop=True)
            nc.tensor.matmul(out=pv[:, 512:H], lhsT=ones[:],
                             rhs=v0[:, 512:H], start=True, stop=True)
            vt = cpool.tile([P, H], mybir.dt.float32)
            nc.scalar.copy(out=vt[:], in_=pv[:])
            Es = [cpool.tile([P, ECW], mybir.dt.float32, tag=f"E{k}",
                             name=f"E{k}") for k in range(E_CHUNKS)]
            b0 = 0
            for d, nb in enumerate(DMA_SCHED):
                t = spool.tile([P, nb, H], mybir.dt.float32, tag=f"t{nb}")
                rows = enc.ap()[b0 * P:(b0 + nb) * P, :]
                src = rows.rearrange("(i p) h -> p i h", p=P)
                if d < N_SPLIT_HEAD:
                    # fan the first transfers across 4 queues for fast rampup
                    for q in range(4):
                        nc.sync.dma_start(out=t[q * 32:(q + 1) * 32],
                                          in_=src[q * 32:(q + 1) * 32])
                else:
                    nc.sync.dma_start(out=t[:], in_=src)
                for i in range(nb):
                    b = b0 + i
                    prod = ppool.tile([P, H], mybir.dt.float32, tag="prod")
                    nc.vector.tensor_tensor(out=prod[:], in0=t[:, i, :],
                                            in1=vt[:],
                                            op=mybir.AluOpType.mult)
                    Et, col = Es[b // ECW], b % ECW
                    cp = opool.tile([P, H], mybir.dt.float32, tag="cp")
                    nc.scalar.activation(
                        out=cp[:], in_=prod[:],
                        func=mybir.ActivationFunctionType.Copy,
                        accum_out=Et[:, col:col + 1])
                b0 += nb
            for k in range(E_CHUNKS):
                nc.sync.dma_start(out=e_out.ap()[:, k * ECW:(k + 1) * ECW],
                                  in_=Es[k][:])
    nc.compile()
    return nc


def _get_nc():
    if "nc" not in _cache:
        _cache["nc"] = _build()
    return _cache["nc"]


def kernel(hidden, encoder_outputs, W, b):
    from concourse import bass_utils

    nc = _get_nc()
    h = np.asarray(hidden, dtype=np.float32)[0]
    enc = np.ascontiguousarray(np.asarray(encoder_outputs,
                                          dtype=np.float32)[:, 0, :])
    v = (np.asarray(W, dtype=np.float32).T @ h).astype(np.float32)

    in_maps = [{"enc": enc[c * S_SHARD:(c + 1) * S_SHARD],
                "v_in": v[None, :]} for c in range(N_CORES)]
    res = bass_utils.run_bass_kernel_spmd(
        nc, in_maps, core_ids=list(range(N_CORES)),
        trace=_cache.get("trace", False))
    _cache["last_result"] = res

    # e_out is [partition, block]; global row s = core*4096 + block*128 + p.
    e = np.concatenate(
        [res.results[c]["e_out"].T.reshape(S_SHARD) for c in range(N_CORES)])
    e = e - e.max()
    p = np.exp(e)
    out = (p / p.sum()).astype(np.float32)
    return out[None, None, :]


# revision 18
# speedup vs baseline: 1.0945x; 1.0945x over previous
"""Trainium2 Bass kernel for nn_Attn: softmax(enc @ (W^T h)) over seq_len.

Math: energy = enc @ W^T + b; attn = energy @ h; out = softmax(attn).
Algebraically attn[s] = enc[s,:] . v + (b.h) with v = W^T h, and the (b.h)
term is constant across s so softmax cancels it. The device work is the
memory-bound part: streaming the 128 MiB encoder_outputs once, sharded
along seq_len across 8 NeuronCores. Per 128-row block: VectorE multiplies
by v (tensor_tensor), ScalarE reduces rows (activation Copy + accum_out),
so the two passes over the data run on different engines concurrently.
"""
import numpy as np

S = 32768
H = 1024
N_CORES = 8
S_SHARD = S // N_CORES          # 4096 rows per core
P = 128                         # partitions
N_BLK = S_SHARD // P            # 32 row-blocks per core
# blocks per dma_start: small at the edges (fast pipeline rampup/drain),
# 2 MiB in the middle (DMA efficiency); must sum to N_BLK
DMA_SCHED = [1, 1] + [2] * 14 + [1, 1]
N_SPLIT_HEAD = 0                # first DMAs split 4-way across queues
E_CHUNKS = 4                    # output DMA'd in column chunks as it completes

_cache = {}


def _build():
    from concourse import bacc, mybir, tile

    nc = bacc.Bacc("TRN2", target_bir_lowering=False, debug=False,
                   num_devices=N_CORES)
    enc = nc.dram_tensor("enc", [S_SHARD, H], mybir.dt.float32,
                         kind="ExternalInput")
    vrep = nc.dram_tensor("vrep", [P, H], mybir.dt.float32,
                          kind="ExternalInput")
    e_out = nc.dram_tensor("e_out", [P, N_BLK], mybir.dt.float32,
                           kind="ExternalOutput")

    ECW = N_BLK // E_CHUNKS     # columns per output chunk

    with tile.TileContext(nc) as tc:
        with tc.tile_pool(name="const", bufs=1) as cpool, \
             tc.tile_pool(name="stream", bufs=8) as spool, \
             tc.tile_pool(name="prod", bufs=4) as ppool, \
             tc.tile_pool(name="cpout", bufs=4) as opool:
            vt = cpool.tile([P, H], mybir.dt.float32)
            nc.sync.dma_start(out=vt[:], in_=vrep.ap())
            Es = [cpool.tile([P, ECW], mybir.dt.float32, tag=f"E{k}",
                             name=f"E{k}") for k in range(E_CHUNKS)]
            b0 = 0
            for d, nb in enumerate(DMA_SCHED):
                t = spool.tile([P, nb, H], mybir.dt.float32, tag=f"t{nb}")
                rows = enc.ap()[b0 * P:(b0 + nb) * P, :]
                src = rows.rearrange("(i p) h -> p i h", p=P)
                if d < N_SPLIT_HEAD:
                    # fan the first transfers across 4 queues for fast rampup
                    for q in range(4):
                        nc.sync.dma_start(out=t[q * 32:(q + 1) * 32],
                                          in_=src[q * 32:(q + 1) * 32])
                else:
                    nc.sync.dma_start(out=t[:], in_=src)
                for i in range(nb):
                    b = b0 + i
                    prod = ppool.tile([P, H], mybir.dt.float32, tag="prod")
                    nc.vector.tensor_tensor(out=prod[:], in0=t[:, i, :],
                                            in1=vt[:],
                                            op=mybir.AluOpType.mult)
                    Et, col = Es[b // ECW], b % ECW
                    cp = opool.tile([P, H], mybir.dt.float32, tag="cp")
                    nc.scalar.activation(
                        out=cp[:], in_=prod[:],
                        func=mybir.ActivationFunctionType.Copy,
                        accum_out=Et[:, col:col + 1])
                b0 += nb
            for k in range(E_CHUNKS):
                nc.sync.dma_start(out=e_out.ap()[:, k * ECW:(k + 1) * ECW],
                                  in_=Es[k][:])
    nc.compile()
    return nc


def _get_nc():
    if "nc" not in _cache:
        _cache["nc"] = _build()
    return _cache["nc"]


def kernel(hidden, encoder_outputs, W, b):
    from concourse import bass_utils

    nc = _get_nc()
    h = np.asarray(hidden, dtype=np.float32)[0]
    enc = np.ascontiguousarray(np.asarray(encoder_outputs,
                                          dtype=np.float32)[:, 0, :])
    v = (np.asarray(W, dtype=np.float32).T @ h).astype(np.float32)
    vrep = np.ascontiguousarray(np.broadcast_to(v[None, :], (P, H)))

    in_maps = [{"enc": enc[c * S_SHARD:(c + 1) * S_SHARD],
                "vrep": vrep} for c in range(N_CORES)]
    res = bass_utils.run_bass_kernel_spmd(
        nc, in_maps, core_ids=list(range(N_CORES)),
        trace=_cache.get("trace", False))
    _cache["last_result"] = res

    # e_out is [partition, block]; global row s = core*4096 + block*128 + p.
    e = np.concatenate(
        [res.results[c]["e_out"].T.reshape(S_SHARD) for c in range(N_CORES)])
    e = e - e.max()
    p = np.exp(e)
    out = (p / p.sum()).astype(np.float32)
    return out[None, None, :]
